# revision 15
# baseline (speedup 1.0000x reference)
"""Causal masked-softmax attention-weight kernel for Trainium2 (8 NeuronCores).

Computes, for query/key of shape [B=2, S=2048, H=16, D=64]:
    w = softmax(where(causal_mask, (Q/sqrt(D)) @ K^T, -inf))  -> [B, H, S, S]

Sharding: the 32 (b, h) pairs are split 4-per-core across 8 cores (data
parallel on B, tensor parallel on H). No cross-core communication.

The host pre-transposes Q/K to [heads, D, S] so the device kernel needs no
on-chip transposes: D lands on SBUF partitions, exactly the matmul
contraction layout.  The 128x128 triangular additive mask is supplied as a
tiny host input (avoids any gpsimd work; the Q7 spin-up was on the critical
path).

Per-core Bass/Tile kernel, per head:
  - DMA K^T / Q^T in [64, 512] chunks via HWDGE (line rate), cast f32->f32r
    on DVE (f32r matmuls stream 2x faster than f32).
  - For q-tile i (128 rows): matmul only the causally-needed k range
    (ncols = 128*(i+1), in N=512 chunks) into one PSUM tile [128, 2048],
    add the triangular -1e9 mask on the diagonal 128x128 block (DVE),
    exp (scale=1/8) on ACT with per-row accumulated sums, reciprocal +
    normalize on DVE, DMA the lower-triangle rows to DRAM.  The
    strictly-upper region is never written: the PJRT run path donates
    pre-zeroed output buffers.
"""

import math
from contextlib import ExitStack

import numpy as np

B, S, H, D = 2, 2048, 16, 64
N_CORES = 8
HPC = (B * H) // N_CORES  # heads (b,h pairs) per core
P = 128  # partitions / q-tile rows
NQT = S // P  # q tiles per head
NCH = S // 512  # 512-col chunks per head
MASK_VAL = -1e9

# matmul operand dtype: "f32" (exact, 4 cyc/row), "f32r" (1 cyc/row, reduced
# precision), "bf16"
MM_DTYPE = "f32r"

_compiled = None


def _build(reps=1):
    import concourse.tile as tile
    from concourse import bacc, mybir

    f32 = mybir.dt.float32

    nc = bacc.Bacc(
        "TRN2",
        target_bir_lowering=False,
        debug=False,
        enable_asserts=False,
        num_devices=N_CORES,
    )
    if MM_DTYPE == "f32r":
        mm_dt = mybir.dt.float32r
    elif MM_DTYPE == "bf16":
        mm_dt = mybir.dt.bfloat16
    else:
        mm_dt = f32

    # host supplies pre-transposed [heads, D, S]
    qT_dram = nc.dram_tensor("qT", [HPC, D, S], f32, kind="ExternalInput").ap()
    kT_dram = nc.dram_tensor("kT", [HPC, D, S], f32, kind="ExternalInput").ap()
    cm_dram = nc.dram_tensor("cm", [P, P], f32, kind="ExternalInput").ap()
    out_dram = nc.dram_tensor("out", [HPC, S, S], f32, kind="ExternalOutput").ap()

    with tile.TileContext(nc) as tc, ExitStack() as ctx:
        consts = ctx.enter_context(tc.tile_pool(name="consts", bufs=1))
        ld_pool = ctx.enter_context(tc.tile_pool(name="ld", bufs=2 * NCH))
        kt_pool = ctx.enter_context(tc.tile_pool(name="kt", bufs=2 * NCH))
        qt_pool = ctx.enter_context(tc.tile_pool(name="qt", bufs=2 * NCH))
        p_pool = ctx.enter_context(tc.tile_pool(name="p", bufs=6))
        st_pool = ctx.enter_context(tc.tile_pool(name="st", bufs=8))
        ps_pool = ctx.enter_context(tc.tile_pool(name="ps", bufs=2, space="PSUM"))

        cmask = consts.tile([P, P], dtype=f32)
        nc.sync.dma_start(cmask[:], cm_dram)

        def load_chunk(src_dram, dst_pool, tag, c):
            """Load one [D, 512] chunk of a [D, S] head slab, cast to mm_dt."""
            sl = src_dram[:, c * 512 : (c + 1) * 512]
            if mm_dt == f32:
                t = dst_pool.tile([D, 512], dtype=f32, tag=tag)
                nc.sync.dma_start(t[:], sl)
            else:
                raw = ld_pool.tile([D, 512], dtype=f32, tag="ld")
                nc.sync.dma_start(raw[:], sl)
                t = dst_pool.tile([D, 512], dtype=mm_dt, tag=tag)
                nc.vector.tensor_copy(t[:], raw[:])
            return t

        rep_ctx = tc.For_i(0, reps, 1) if reps > 1 else None
        if rep_ctx is not None:
            ctx.enter_context(rep_ctx)

        for j in range(HPC):
            # interleave so earliest-needed chunks arrive first
            ktc = []
            qtc = []
            for c in range(NCH):
                qtc.append(load_chunk(qT_dram[j], qt_pool, "qt", c))
                ktc.append(load_chunk(kT_dram[j], kt_pool, "kt", c))

            for i in range(NQT):
                ncols = (i + 1) * P
                ql = qtc[i // 4][:, (i % 4) * P : (i % 4 + 1) * P]
                ps = ps_pool.tile([P, S], dtype=f32, tag="ps")
                for m in range(math.ceil(ncols / 512)):
                    nc.tensor.matmul(
                        ps[:, m * 512 : (m + 1) * 512],
                        ql,
                        ktc[m][:],
                        start=True,
                        stop=True,
                    )
                # diagonal 128x128 block: triangular additive mask
                nc.vector.tensor_add(
                    ps[:, i * P : (i + 1) * P], ps[:, i * P : (i + 1) * P], cmask[:]
                )
                p = p_pool.tile([P, S], dtype=f32, tag="p")
                sums = st_pool.tile([P, 1], dtype=f32, tag="sums")
                nc.scalar.activation(
                    p[:, :ncols],
                    ps[:, :ncols],
                    mybir.ActivationFunctionType.Exp,
                    bias=0.0,
                    scale=1.0 / math.sqrt(D),
                    accum_out=sums[:],
                )
                r = st_pool.tile([P, 1], dtype=f32, tag="r")
                nc.vector.reciprocal(r[:], sums[:])
                nc.vector.tensor_scalar_mul(p[:, :ncols], p[:, :ncols], r[:])
                nc.sync.dma_start(
                    out_dram[j, i * P : (i + 1) * P, 0:ncols], p[:, :ncols]
                )

    nc.compile()
    return nc


def _get_compiled():
    global _compiled
    if _compiled is None:
        _compiled = _build()
    return _compiled


def _make_cmask():
    cm = np.zeros((P, P), dtype=np.float32)
    cm[np.triu_indices(P, 1)] = MASK_VAL
    return cm


def _run(query, key, **spmd_kwargs):
    from concourse import bass_utils

    query = np.asarray(query, dtype=np.float32)
    key = np.asarray(key, dtype=np.float32)
    # [B, S, H, D] -> [B*H, D, S]
    qb = np.ascontiguousarray(np.transpose(query, (0, 2, 3, 1)).reshape(B * H, D, S))
    kb = np.ascontiguousarray(np.transpose(key, (0, 2, 3, 1)).reshape(B * H, D, S))
    cm = _make_cmask()
    in_maps = [
        {
            "qT": qb[c * HPC : (c + 1) * HPC],
            "kT": kb[c * HPC : (c + 1) * HPC],
            "cm": cm,
        }
        for c in range(N_CORES)
    ]
    nc = _get_compiled()
    res = bass_utils.run_bass_kernel_spmd(
        nc, in_maps, core_ids=list(range(N_CORES)), **spmd_kwargs
    )
    outs = [r["out"] for r in res.results]
    return np.concatenate(outs, axis=0).reshape(B, H, S, S), res


def kernel(query, key, mask=None):
    """Full-input entry point: query/key [B, S, H, D] f32, mask ignored
    (always the causal tril).  Returns [B, H, S, S] f32."""
    return _run(query, key)[0]


# revision 17
# speedup vs baseline: 1.0576x; 1.0576x over previous
"""Causal masked-softmax attention-weight kernel for Trainium2 (8 NeuronCores).

Computes, for query/key of shape [B=2, S=2048, H=16, D=64]:
    w = softmax(where(causal_mask, (Q/sqrt(D)) @ K^T, -inf))  -> [B, H, S, S]

Sharding: the 32 (b, h) pairs are split 4-per-core across 8 cores (data
parallel on B, tensor parallel on H). No cross-core communication.

The host pre-transposes Q/K to [heads, D, S] so the device kernel needs no
on-chip transposes: D lands on SBUF partitions, exactly the matmul
contraction layout.  The 128x128 triangular additive mask is supplied as a
tiny host input (avoids any gpsimd work; the Q7 spin-up was on the critical
path).

Per-core Bass/Tile kernel, per head:
  - DMA K^T / Q^T in [64, 512] chunks via HWDGE (line rate), cast f32->f32r
    on DVE (f32r matmuls stream 2x faster than f32).
  - For q-tile i (128 rows): matmul only the causally-needed k range
    (ncols = 128*(i+1), in N=512 chunks) into one PSUM tile [128, 2048],
    add the triangular -1e9 mask on the diagonal 128x128 block (DVE),
    exp (scale=1/8) on ACT with per-row accumulated sums, reciprocal +
    normalize on DVE, DMA the lower-triangle rows to DRAM.  The
    strictly-upper region is never written: the PJRT run path donates
    pre-zeroed output buffers.
"""

import math
from contextlib import ExitStack

import numpy as np

B, S, H, D = 2, 2048, 16, 64
N_CORES = 8
HPC = (B * H) // N_CORES  # heads (b,h pairs) per core
P = 128  # partitions / q-tile rows
NQT = S // P  # q tiles per head
NCH = S // 512  # 512-col chunks per head
MASK_VAL = -1e9

# matmul operand dtype: "f32" (exact, 4 cyc/row), "f32r" (1 cyc/row, reduced
# precision), "bf16"
MM_DTYPE = "f32r"

_compiled = None


def _build(reps=1):
    import concourse.tile as tile
    from concourse import bacc, mybir

    f32 = mybir.dt.float32

    nc = bacc.Bacc(
        "TRN2",
        target_bir_lowering=False,
        debug=False,
        enable_asserts=False,
        num_devices=N_CORES,
    )
    if MM_DTYPE == "f32r":
        mm_dt = mybir.dt.float32r
    elif MM_DTYPE == "bf16":
        mm_dt = mybir.dt.bfloat16
    else:
        mm_dt = f32

    # host supplies pre-transposed [heads, D, S]
    qT_dram = nc.dram_tensor("qT", [HPC, D, S], f32, kind="ExternalInput").ap()
    kT_dram = nc.dram_tensor("kT", [HPC, D, S], f32, kind="ExternalInput").ap()
    cm_dram = nc.dram_tensor("cm", [P, P], f32, kind="ExternalInput").ap()
    out_dram = nc.dram_tensor("out", [HPC, S, S], f32, kind="ExternalOutput").ap()

    with tile.TileContext(nc) as tc, ExitStack() as ctx:
        consts = ctx.enter_context(tc.tile_pool(name="consts", bufs=1))
        ld_pool = ctx.enter_context(tc.tile_pool(name="ld", bufs=2))
        kt_pool = ctx.enter_context(tc.tile_pool(name="kt", bufs=2))
        qt_pool = ctx.enter_context(tc.tile_pool(name="qt", bufs=2))
        p_pool = ctx.enter_context(tc.tile_pool(name="p", bufs=6))
        st_pool = ctx.enter_context(tc.tile_pool(name="st", bufs=8))
        ps_pool = ctx.enter_context(tc.tile_pool(name="ps", bufs=2, space="PSUM"))

        cmask = consts.tile([P, P], dtype=f32)
        nc.sync.dma_start(cmask[:], cm_dram)

        def load_early_chunk(src_dram, tag, c):
            """HWDGE f32 load of one [D, 512] chunk + DVE cast to mm_dt.
            Fast-start path for head 0 only (bypasses the SWDGE/Q7 spin-up)."""
            sl = src_dram[:, c * 512 : (c + 1) * 512]
            if mm_dt == f32:
                t = ld_pool.tile([D, 512], dtype=f32, tag=tag)
                nc.sync.dma_start(t[:], sl)
                return t
            raw = ld_pool.tile([D, 512], dtype=f32, tag="lde")
            nc.sync.dma_start(raw[:], sl)
            t = ld_pool.tile([D, 512], dtype=mm_dt, tag=tag)
            nc.vector.tensor_copy(t[:], raw[:])
            return t

        # casting loads (f32 -> f32r/bf16) must go through SWDGE (gpsimd)
        load_engine = nc.sync if mm_dt == f32 else nc.gpsimd

        rep_ctx = tc.For_i(0, reps, 1) if reps > 1 else None
        if rep_ctx is not None:
            ctx.enter_context(rep_ctx)

        # early chunks covering head 0, q-tiles 0..7
        qte = [load_early_chunk(qT_dram[0], "qte", c) for c in range(2)]
        kte = [load_early_chunk(kT_dram[0], "kte", c) for c in range(2)]

        for j in range(HPC):
            kt = kt_pool.tile([D, S], dtype=mm_dt, tag="kt")
            load_engine.dma_start(kt[:], kT_dram[j])
            qt = qt_pool.tile([D, S], dtype=mm_dt, tag="qt")
            load_engine.dma_start(qt[:], qT_dram[j])

            for i in range(NQT):
                ncols = (i + 1) * P
                early = j == 0 and i < 8
                if early:
                    ql = qte[i // 4][:, (i % 4) * P : (i % 4 + 1) * P]
                else:
                    ql = qt[:, i * P : (i + 1) * P]
                ps = ps_pool.tile([P, S], dtype=f32, tag="ps")
                for m in range(math.ceil(ncols / 512)):
                    nc.tensor.matmul(
                        ps[:, m * 512 : (m + 1) * 512],
                        ql,
                        kte[m][:] if early else kt[:, m * 512 : (m + 1) * 512],
                        start=True,
                        stop=True,
                    )
                # diagonal 128x128 block: triangular additive mask
                nc.vector.tensor_add(
                    ps[:, i * P : (i + 1) * P], ps[:, i * P : (i + 1) * P], cmask[:]
                )
                p = p_pool.tile([P, S], dtype=f32, tag="p")
                sums = st_pool.tile([P, 1], dtype=f32, tag="sums")
                nc.scalar.activation(
                    p[:, :ncols],
                    ps[:, :ncols],
                    mybir.ActivationFunctionType.Exp,
                    bias=0.0,
                    scale=1.0 / math.sqrt(D),
                    accum_out=sums[:],
                )
                r = st_pool.tile([P, 1], dtype=f32, tag="r")
                nc.vector.reciprocal(r[:], sums[:])
                nc.vector.tensor_scalar_mul(p[:, :ncols], p[:, :ncols], r[:])
                nc.sync.dma_start(
                    out_dram[j, i * P : (i + 1) * P, 0:ncols], p[:, :ncols]
                )

    nc.compile()
    return nc


def _get_compiled():
    global _compiled
    if _compiled is None:
        _compiled = _build()
    return _compiled


def _make_cmask():
    cm = np.zeros((P, P), dtype=np.float32)
    cm[np.triu_indices(P, 1)] = MASK_VAL
    return cm


def _run(query, key, **spmd_kwargs):
    from concourse import bass_utils

    query = np.asarray(query, dtype=np.float32)
    key = np.asarray(key, dtype=np.float32)
    # [B, S, H, D] -> [B*H, D, S]
    qb = np.ascontiguousarray(np.transpose(query, (0, 2, 3, 1)).reshape(B * H, D, S))
    kb = np.ascontiguousarray(np.transpose(key, (0, 2, 3, 1)).reshape(B * H, D, S))
    cm = _make_cmask()
    in_maps = [
        {
            "qT": qb[c * HPC : (c + 1) * HPC],
            "kT": kb[c * HPC : (c + 1) * HPC],
            "cm": cm,
        }
        for c in range(N_CORES)
    ]
    nc = _get_compiled()
    res = bass_utils.run_bass_kernel_spmd(
        nc, in_maps, core_ids=list(range(N_CORES)), **spmd_kwargs
    )
    outs = [r["out"] for r in res.results]
    return np.concatenate(outs, axis=0).reshape(B, H, S, S), res


def kernel(query, key, mask=None):
    """Full-input entry point: query/key [B, S, H, D] f32, mask ignored
    (always the causal tril).  Returns [B, H, S, S] f32."""
    return _run(query, key)[0]


# revision 19
# speedup vs baseline: 1.0840x; 1.0250x over previous
"""Causal masked-softmax attention-weight kernel for Trainium2 (8 NeuronCores).

Computes, for query/key of shape [B=2, S=2048, H=16, D=64]:
    w = softmax(where(causal_mask, (Q/sqrt(D)) @ K^T, -inf))  -> [B, H, S, S]

Sharding: the 32 (b, h) pairs are split 4-per-core across 8 cores (data
parallel on B, tensor parallel on H). No cross-core communication.

The host pre-transposes Q/K to [heads, D, S] so the device kernel needs no
on-chip transposes: D lands on SBUF partitions, exactly the matmul
contraction layout.  The 128x128 triangular additive mask is supplied as a
tiny host input (avoids any gpsimd work; the Q7 spin-up was on the critical
path).

Per-core Bass/Tile kernel, per head:
  - DMA K^T / Q^T in [64, 512] chunks via HWDGE (line rate), cast f32->f32r
    on DVE (f32r matmuls stream 2x faster than f32).
  - For q-tile i (128 rows): matmul only the causally-needed k range
    (ncols = 128*(i+1), in N=512 chunks) into one PSUM tile [128, 2048],
    add the triangular -1e9 mask on the diagonal 128x128 block (DVE),
    exp (scale=1/8) on ACT with per-row accumulated sums, reciprocal +
    normalize on DVE, DMA the lower-triangle rows to DRAM.  The
    strictly-upper region is never written: the PJRT run path donates
    pre-zeroed output buffers.
"""

import math
from contextlib import ExitStack

import numpy as np

B, S, H, D = 2, 2048, 16, 64
N_CORES = 8
HPC = (B * H) // N_CORES  # heads (b,h pairs) per core
P = 128  # partitions / q-tile rows
NQT = S // P  # q tiles per head
NCH = S // 512  # 512-col chunks per head
MASK_VAL = -1e9

# matmul operand dtype: "f32" (exact, 4 cyc/row), "f32r" (1 cyc/row, reduced
# precision), "bf16"
MM_DTYPE = "f32r"

_compiled = None


def _build(reps=1):
    import concourse.tile as tile
    from concourse import bacc, mybir

    f32 = mybir.dt.float32

    nc = bacc.Bacc(
        "TRN2",
        target_bir_lowering=False,
        debug=False,
        enable_asserts=False,
        num_devices=N_CORES,
    )
    if MM_DTYPE == "f32r":
        mm_dt = mybir.dt.float32r
    elif MM_DTYPE == "bf16":
        mm_dt = mybir.dt.bfloat16
    else:
        mm_dt = f32

    # host supplies pre-transposed [heads, D, S]
    qT_dram = nc.dram_tensor("qT", [HPC, D, S], f32, kind="ExternalInput").ap()
    kT_dram = nc.dram_tensor("kT", [HPC, D, S], f32, kind="ExternalInput").ap()
    cm_dram = nc.dram_tensor("cm", [P, P], f32, kind="ExternalInput").ap()
    out_dram = nc.dram_tensor("out", [HPC, S, S], f32, kind="ExternalOutput").ap()

    with tile.TileContext(nc) as tc, ExitStack() as ctx:
        consts = ctx.enter_context(tc.tile_pool(name="consts", bufs=1))
        ld_pool = ctx.enter_context(tc.tile_pool(name="ld", bufs=8))
        kt_pool = ctx.enter_context(tc.tile_pool(name="kt", bufs=2))
        qt_pool = ctx.enter_context(tc.tile_pool(name="qt", bufs=2))
        p_pool = ctx.enter_context(tc.tile_pool(name="p", bufs=6))
        st_pool = ctx.enter_context(tc.tile_pool(name="st", bufs=8))
        ps_pool = ctx.enter_context(tc.tile_pool(name="ps", bufs=2, space="PSUM"))

        cmask = consts.tile([P, P], dtype=f32)
        nc.sync.dma_start(cmask[:], cm_dram)

        def load_early_chunk(src_dram, tag, c):
            """HWDGE f32 load of one [D, 512] chunk + DVE cast to mm_dt.
            Fast-start path for head 0 only (bypasses the SWDGE/Q7 spin-up)."""
            sl = src_dram[:, c * 512 : (c + 1) * 512]
            if mm_dt == f32:
                t = ld_pool.tile([D, 512], dtype=f32, tag=tag)
                nc.sync.dma_start(t[:], sl)
                return t
            raw = ld_pool.tile([D, 512], dtype=f32, tag="lde")
            nc.sync.dma_start(raw[:], sl)
            t = ld_pool.tile([D, 512], dtype=mm_dt, tag=tag)
            nc.vector.tensor_copy(t[:], raw[:])
            return t

        # casting loads (f32 -> f32r/bf16) must go through SWDGE (gpsimd)
        load_engine = nc.sync if mm_dt == f32 else nc.gpsimd

        # warm the ACT exp table off the critical path
        warm = st_pool.tile([P, 1], dtype=f32, tag="warm")
        nc.vector.memset(warm[:], 0.0)
        nc.scalar.activation(
            warm[:], warm[:], mybir.ActivationFunctionType.Exp, bias=0.0, scale=1.0
        )

        rep_ctx = tc.For_i(0, reps, 1) if reps > 1 else None
        if rep_ctx is not None:
            ctx.enter_context(rep_ctx)

        # head 0 entirely via fast-start HWDGE chunks, earliest-needed first
        qte = {}
        kte = {}
        for c in [0, 1, 2, 3]:
            qte[c] = load_early_chunk(qT_dram[0], "qte", c)
            kte[c] = load_early_chunk(kT_dram[0], "kte", c)

        for j in range(HPC):
            if j > 0:
                kt = kt_pool.tile([D, S], dtype=mm_dt, tag="kt")
                load_engine.dma_start(kt[:], kT_dram[j])
                qt = qt_pool.tile([D, S], dtype=mm_dt, tag="qt")
                load_engine.dma_start(qt[:], qT_dram[j])

            for i in range(NQT):
                ncols = (i + 1) * P
                if j == 0:
                    ql = qte[i // 4][:, (i % 4) * P : (i % 4 + 1) * P]
                else:
                    ql = qt[:, i * P : (i + 1) * P]
                ps = ps_pool.tile([P, S], dtype=f32, tag="ps")
                for m in range(math.ceil(ncols / 512)):
                    nc.tensor.matmul(
                        ps[:, m * 512 : (m + 1) * 512],
                        ql,
                        kte[m][:] if j == 0 else kt[:, m * 512 : (m + 1) * 512],
                        start=True,
                        stop=True,
                    )
                # diagonal 128x128 block: triangular additive mask
                nc.vector.tensor_add(
                    ps[:, i * P : (i + 1) * P], ps[:, i * P : (i + 1) * P], cmask[:]
                )
                p = p_pool.tile([P, S], dtype=f32, tag="p")
                sums = st_pool.tile([P, 1], dtype=f32, tag="sums")
                nc.scalar.activation(
                    p[:, :ncols],
                    ps[:, :ncols],
                    mybir.ActivationFunctionType.Exp,
                    bias=0.0,
                    scale=1.0 / math.sqrt(D),
                    accum_out=sums[:],
                )
                r = st_pool.tile([P, 1], dtype=f32, tag="r")
                nc.vector.reciprocal(r[:], sums[:])
                nc.vector.tensor_scalar_mul(p[:, :ncols], p[:, :ncols], r[:])
                nc.sync.dma_start(
                    out_dram[j, i * P : (i + 1) * P, 0:ncols], p[:, :ncols]
                )

    nc.compile()
    return nc


def _get_compiled():
    global _compiled
    if _compiled is None:
        _compiled = _build()
    return _compiled


def _make_cmask():
    cm = np.zeros((P, P), dtype=np.float32)
    cm[np.triu_indices(P, 1)] = MASK_VAL
    return cm


def _run(query, key, **spmd_kwargs):
    from concourse import bass_utils

    query = np.asarray(query, dtype=np.float32)
    key = np.asarray(key, dtype=np.float32)
    # [B, S, H, D] -> [B*H, D, S]
    qb = np.ascontiguousarray(np.transpose(query, (0, 2, 3, 1)).reshape(B * H, D, S))
    kb = np.ascontiguousarray(np.transpose(key, (0, 2, 3, 1)).reshape(B * H, D, S))
    cm = _make_cmask()
    in_maps = [
        {
            "qT": qb[c * HPC : (c + 1) * HPC],
            "kT": kb[c * HPC : (c + 1) * HPC],
            "cm": cm,
        }
        for c in range(N_CORES)
    ]
    nc = _get_compiled()
    res = bass_utils.run_bass_kernel_spmd(
        nc, in_maps, core_ids=list(range(N_CORES)), **spmd_kwargs
    )
    outs = [r["out"] for r in res.results]
    return np.concatenate(outs, axis=0).reshape(B, H, S, S), res


def kernel(query, key, mask=None):
    """Full-input entry point: query/key [B, S, H, D] f32, mask ignored
    (always the causal tril).  Returns [B, H, S, S] f32."""
    return _run(query, key)[0]


# revision 21
# speedup vs baseline: 1.1485x; 1.0595x over previous
"""Causal masked-softmax attention-weight kernel for Trainium2 (8 NeuronCores).

Computes, for query/key of shape [B=2, S=2048, H=16, D=64]:
    w = softmax(where(causal_mask, (Q/sqrt(D)) @ K^T, -inf))  -> [B, H, S, S]

Sharding: the 32 (b, h) pairs are split 4-per-core across 8 cores (data
parallel on B, tensor parallel on H). No cross-core communication.

The host pre-transposes Q/K to [heads, D, S] so the device kernel needs no
on-chip transposes: D lands on SBUF partitions, exactly the matmul
contraction layout.  The 128x128 triangular additive mask is supplied as a
tiny host input (avoids any gpsimd work; the Q7 spin-up was on the critical
path).

Per-core Bass/Tile kernel, per head:
  - DMA K^T / Q^T in [64, 512] chunks via HWDGE (line rate), cast f32->f32r
    on DVE (f32r matmuls stream 2x faster than f32).
  - For q-tile i (128 rows): matmul only the causally-needed k range
    (ncols = 128*(i+1), in N=512 chunks) into one PSUM tile [128, 2048],
    add the triangular -1e9 mask on the diagonal 128x128 block (DVE),
    exp (scale=1/8) on ACT with per-row accumulated sums, reciprocal +
    normalize on DVE, DMA the lower-triangle rows to DRAM.  The
    strictly-upper region is never written: the PJRT run path donates
    pre-zeroed output buffers.
"""

import math
from contextlib import ExitStack

import numpy as np

B, S, H, D = 2, 2048, 16, 64
N_CORES = 8
HPC = (B * H) // N_CORES  # heads (b,h pairs) per core
P = 128  # partitions / q-tile rows
NQT = S // P  # q tiles per head
NCH = S // 512  # 512-col chunks per head
MASK_VAL = -1e9

# matmul operand dtype: "f32" (exact, 4 cyc/row), "f32r" (1 cyc/row, reduced
# precision), "bf16"
MM_DTYPE = "f32r"

_compiled = None


def _build(reps=1):
    import concourse.tile as tile
    from concourse import bacc, mybir

    f32 = mybir.dt.float32

    nc = bacc.Bacc(
        "TRN2",
        target_bir_lowering=False,
        debug=False,
        enable_asserts=False,
        num_devices=N_CORES,
    )
    if MM_DTYPE == "f32r":
        mm_dt = mybir.dt.float32r
    elif MM_DTYPE == "bf16":
        mm_dt = mybir.dt.bfloat16
    else:
        mm_dt = f32

    # host supplies pre-transposed [heads, D, S]
    qT_dram = nc.dram_tensor("qT", [HPC, D, S], f32, kind="ExternalInput").ap()
    kT_dram = nc.dram_tensor("kT", [HPC, D, S], f32, kind="ExternalInput").ap()
    cm_dram = nc.dram_tensor("cm", [P, P], f32, kind="ExternalInput").ap()
    out_dram = nc.dram_tensor("out", [HPC, S, S], f32, kind="ExternalOutput").ap()

    with tile.TileContext(nc) as tc, ExitStack() as ctx:
        consts = ctx.enter_context(tc.tile_pool(name="consts", bufs=1))
        kt_pool = ctx.enter_context(tc.tile_pool(name="kt", bufs=2 * NCH))
        qt_pool = ctx.enter_context(tc.tile_pool(name="qt", bufs=2 * NCH))
        p_pool = ctx.enter_context(tc.tile_pool(name="p", bufs=6))
        st_pool = ctx.enter_context(tc.tile_pool(name="st", bufs=8))
        ps_pool = ctx.enter_context(tc.tile_pool(name="ps", bufs=2, space="PSUM"))

        cmask = consts.tile([P, P], dtype=f32)
        nc.sync.dma_start(cmask[:], cm_dram)

        # casting loads (f32 -> f32r/bf16) must go through SWDGE (gpsimd);
        # SWDGE also keeps reads off the HWDGE queues that stream the output
        # writes (sharing those FIFOs measurably degrades write bandwidth)
        load_engine = nc.sync if mm_dt == f32 else nc.gpsimd

        # warm the ACT exp table off the critical path
        warm = st_pool.tile([P, 1], dtype=f32, tag="warm")
        nc.vector.memset(warm[:], 0.0)
        nc.scalar.activation(
            warm[:], warm[:], mybir.ActivationFunctionType.Exp, bias=0.0, scale=1.0
        )

        rep_ctx = tc.For_i(0, reps, 1) if reps > 1 else None
        if rep_ctx is not None:
            ctx.enter_context(rep_ctx)

        def load_chunk(src_dram, pool, tag, c):
            t = pool.tile([D, 512], dtype=mm_dt, tag=tag)
            load_engine.dma_start(t[:], src_dram[:, c * 512 : (c + 1) * 512])
            return t

        for j in range(HPC):
            # chunked loads, earliest-needed first, so q-tile 0 starts after
            # ~0.25 MB instead of a full 1 MB slab
            qtc = {}
            ktc = {}
            for c in range(NCH):
                qtc[c] = load_chunk(qT_dram[j], qt_pool, "qt", c)
                ktc[c] = load_chunk(kT_dram[j], kt_pool, "kt", c)

            for i in range(NQT):
                ncols = (i + 1) * P
                ql = qtc[i // 4][:, (i % 4) * P : (i % 4 + 1) * P]
                ps = ps_pool.tile([P, S], dtype=f32, tag="ps")
                for m in range(math.ceil(ncols / 512)):
                    nc.tensor.matmul(
                        ps[:, m * 512 : (m + 1) * 512],
                        ql,
                        ktc[m][:],
                        start=True,
                        stop=True,
                    )
                # diagonal 128x128 block: triangular additive mask
                nc.vector.tensor_add(
                    ps[:, i * P : (i + 1) * P], ps[:, i * P : (i + 1) * P], cmask[:]
                )
                p = p_pool.tile([P, S], dtype=f32, tag="p")
                sums = st_pool.tile([P, 1], dtype=f32, tag="sums")
                nc.scalar.activation(
                    p[:, :ncols],
                    ps[:, :ncols],
                    mybir.ActivationFunctionType.Exp,
                    bias=0.0,
                    scale=1.0 / math.sqrt(D),
                    accum_out=sums[:],
                )
                r = st_pool.tile([P, 1], dtype=f32, tag="r")
                nc.vector.reciprocal(r[:], sums[:])
                nc.vector.tensor_scalar_mul(p[:, :ncols], p[:, :ncols], r[:])
                nc.sync.dma_start(
                    out_dram[j, i * P : (i + 1) * P, 0:ncols], p[:, :ncols]
                )

    nc.compile()
    return nc


def _get_compiled():
    global _compiled
    if _compiled is None:
        _compiled = _build()
    return _compiled


def _make_cmask():
    cm = np.zeros((P, P), dtype=np.float32)
    cm[np.triu_indices(P, 1)] = MASK_VAL
    return cm


def _run(query, key, **spmd_kwargs):
    from concourse import bass_utils

    query = np.asarray(query, dtype=np.float32)
    key = np.asarray(key, dtype=np.float32)
    # [B, S, H, D] -> [B*H, D, S]
    qb = np.ascontiguousarray(np.transpose(query, (0, 2, 3, 1)).reshape(B * H, D, S))
    kb = np.ascontiguousarray(np.transpose(key, (0, 2, 3, 1)).reshape(B * H, D, S))
    cm = _make_cmask()
    in_maps = [
        {
            "qT": qb[c * HPC : (c + 1) * HPC],
            "kT": kb[c * HPC : (c + 1) * HPC],
            "cm": cm,
        }
        for c in range(N_CORES)
    ]
    nc = _get_compiled()
    res = bass_utils.run_bass_kernel_spmd(
        nc, in_maps, core_ids=list(range(N_CORES)), **spmd_kwargs
    )
    outs = [r["out"] for r in res.results]
    return np.concatenate(outs, axis=0).reshape(B, H, S, S), res


def kernel(query, key, mask=None):
    """Full-input entry point: query/key [B, S, H, D] f32, mask ignored
    (always the causal tril).  Returns [B, H, S, S] f32."""
    return _run(query, key)[0]
